# revision 1
# baseline (speedup 1.0000x reference)
"""LATTE GNN forward on 8 Trainium2 NeuronCores.

Math: the reference's per-edge message is v[dst] (the destination node's own
projected feature), and segment-softmax weights over each destination's
incoming edges sum to exactly 1.  Hence the edge aggregation reduces to
    h_m[n] = v[n] * mask_m[n],   mask_m[n] = [node n has >=1 incoming edge in rel m]
and the whole module collapses to
    v      = feat @ Wr + br                       [N, 256]
    vl[n,h]= v[n,h,:] . rel_attn_l[h]             (= feat @ (Wr @ RLbd) + br.RLbd)
    vr[n,h]= v[n,h,:] . rel_attn_r[h]
    logit[n,r,h] = lrelu(vl + mask_r * vr)
    beta   = softmax over h (axis=2 of [N,M+1,H] in the reference!)
    s[n,h] = sum_r mask_r[n] * beta[n,r,h]        (mask_3 = 1)
    out    = relu(LN(v * s) * gamma + ln_beta)
Node-sharded across 8 cores (rows 6250/core, padded to 6272 = 49*128).
Edge structure enters only through the per-node masks (host bincount).
"""

import numpy as np

N, D, H, C, M = 50000, 256, 4, 64, 3
NCORES = 8
RPC = N // NCORES          # 6250 rows per core
NT = 49                    # 128-row tiles per core
RPAD = NT * 128            # 6272
EPS = 1e-5

_CACHE = {}
LAST_RESULT = None         # BassKernelResults of the most recent run (for test.py)


def _build(trace=False):
    import concourse.bass as bass
    import concourse.mybir as mybir
    from concourse.tile import TileContext

    fp32 = mybir.dt.float32
    AF = mybir.ActivationFunctionType
    OP = mybir.AluOpType

    nc = bass.Bass()
    featT = nc.declare_dram_parameter("featT", [128, 2, RPAD], fp32, isOutput=False)
    constd = nc.declare_dram_parameter("constd", [128, 1628], fp32, isOutput=False)
    out = nc.declare_dram_parameter("out", [RPAD, 256], fp32, isOutput=True)

    with TileContext(nc) as tc:
        with (
            tc.tile_pool(name="const", bufs=1) as cpool,
            tc.tile_pool(name="ft", bufs=4) as ftpool,
            tc.tile_pool(name="small", bufs=4) as spool,
            tc.tile_pool(name="big", bufs=3) as bpool,
            tc.tile_pool(name="psv", bufs=2, space="PSUM") as pvpool,
            tc.tile_pool(name="pslv", bufs=2, space="PSUM") as plpool,
        ):
            const_sb = cpool.tile([128, 1628], fp32, tag="const")
            nc.gpsimd.dma_start(out=const_sb[:], in_=constd[:])
            # layout: [0:512) Wr k-chunks, [512:528) A k-chunks,
            # [528:784) gamma, [784:1040) beta,
            # row0 [1040:1304) biasrow, row0 [1304:1432) ones,
            # [1432:1628) per-tile masks (tile i -> [1432+4i, 1436+4i))
            gam_sb = const_sb[:, 528:784]
            bet_sb = const_sb[:, 784:1040]
            # dummy matmul: absorbs the const-DMA wait on PE so later
            # matmuls carry only their own ftT-DMA wait (1-wait ISA limit)
            dummy_ps = plpool.tile([128, 1], fp32, tag="lv")
            nc.tensor.matmul(dummy_ps[:], const_sb[0:1, 1304:1432],
                             const_sb[0:1, 1040:1041], start=True, stop=True)

            for i in range(NT):
                r0 = i * 128
                ftT = ftpool.tile([128, 2, 128], fp32, tag="ftT")
                nc.sync.dma_start(out=ftT[:], in_=featT[:, :, r0:r0 + 128])
                mk = const_sb[:, 1432 + 4 * i:1436 + 4 * i]

                # v = feat @ Wr + br    [128 rows, 256]
                v_ps = pvpool.tile([128, 256], fp32, tag="v")
                nc.tensor.matmul(v_ps[:], ftT[:, 0, :], const_sb[:, 0:256], start=True, stop=False)
                nc.tensor.matmul(v_ps[:], ftT[:, 1, :], const_sb[:, 256:512], start=False, stop=False)
                nc.tensor.matmul(v_ps[:], const_sb[0:1, 1304:1432],
                                 const_sb[0:1, 1040:1296], start=False, stop=True)
                # [vl | vr]   [128, 8]
                lv_ps = plpool.tile([128, 8], fp32, tag="lv")
                nc.tensor.matmul(lv_ps[:], ftT[:, 0, :], const_sb[:, 512:520], start=True, stop=False)
                nc.tensor.matmul(lv_ps[:], ftT[:, 1, :], const_sb[:, 520:528], start=False, stop=False)
                nc.tensor.matmul(lv_ps[:], const_sb[0:1, 1304:1432],
                                 const_sb[0:1, 1296:1304], start=False, stop=True)

                mk3 = mk.unsqueeze(2).broadcast_to((128, 4, 4))      # (r,h) r-major
                vl3 = lv_ps[:, 0:4].unsqueeze(1).broadcast_to((128, 4, 4))
                vr3 = lv_ps[:, 4:8].unsqueeze(1).broadcast_to((128, 4, 4))

                lg = spool.tile([128, 16], fp32, tag="lg")
                lg3 = lg[:].rearrange("p (r h) -> p r h", r=4)
                nc.vector.tensor_tensor(out=lg3, in0=mk3, in1=vr3, op=OP.mult)
                nc.vector.tensor_tensor(out=lg3, in0=lg3, in1=vl3, op=OP.add)
                lr = spool.tile([128, 16], fp32, tag="lr")
                # leaky_relu(x) = max(0.2*x, x)
                nc.vector.scalar_tensor_tensor(out=lr[:], in0=lg[:], scalar=0.2,
                                               in1=lg[:], op0=OP.mult, op1=OP.max)
                ext = spool.tile([128, 16], fp32, tag="ext")
                nc.scalar.activation(ext[:], lr[:], AF.Exp)
                ex3 = ext[:].rearrange("p (r h) -> p r h", r=4)
                den = spool.tile([128, 4], fp32, tag="den")
                nc.vector.tensor_reduce(out=den[:], in_=ex3, axis=mybir.AxisListType.X,
                                        op=OP.add)
                rden = spool.tile([128, 4], fp32, tag="rden")
                nc.vector.reciprocal(rden[:], den[:])
                mrd = spool.tile([128, 4], fp32, tag="mrd")
                nc.vector.tensor_tensor(out=mrd[:], in0=mk, in1=rden[:], op=OP.mult)
                wex = spool.tile([128, 16], fp32, tag="wex")
                wex3 = wex[:].rearrange("p (r h) -> p r h", r=4)
                nc.vector.tensor_tensor(out=wex3, in0=ex3,
                                        in1=mrd[:].unsqueeze(2).broadcast_to((128, 4, 4)),
                                        op=OP.mult)
                s4 = spool.tile([128, 4], fp32, tag="s4")
                nc.vector.tensor_reduce(out=s4[:],
                                        in_=wex[:].rearrange("p (r h) -> p h r", r=4),
                                        axis=mybir.AxisListType.X, op=OP.add)

                # o = v * s (broadcast over c), fused row-sum
                o_t = bpool.tile([128, 256], fp32, tag="o")
                sum_t = spool.tile([128, 1], fp32, tag="sum")
                nc.vector.scalar_tensor_tensor(
                    out=o_t[:].rearrange("p (h c) -> p h c", h=4),
                    in0=v_ps[:].rearrange("p (h c) -> p h c", h=4),
                    scalar=1.0, op0=OP.bypass,
                    in1=s4[:].unsqueeze(2).broadcast_to((128, 4, 64)),
                    op1=OP.mult, accum_out=sum_t[:])
                sq_t = bpool.tile([128, 256], fp32, tag="sq")
                ssq = spool.tile([128, 1], fp32, tag="ssq")
                nc.scalar.activation(sq_t[:], o_t[:], AF.Square, accum_out=ssq[:])
                mean = spool.tile([128, 1], fp32, tag="mean")
                nc.scalar.mul(mean[:], sum_t[:], 1.0 / 256.0)
                em2 = spool.tile([128, 1], fp32, tag="em2")
                nc.scalar.mul(em2[:], ssq[:], 1.0 / 256.0)
                m2 = spool.tile([128, 1], fp32, tag="m2")
                nc.vector.tensor_tensor(out=m2[:], in0=mean[:], in1=mean[:], op=OP.mult)
                varr = spool.tile([128, 1], fp32, tag="varr")
                nc.vector.scalar_tensor_tensor(out=varr[:], in0=em2[:], scalar=EPS,
                                               in1=m2[:], op0=OP.add,
                                               op1=OP.subtract)
                std = spool.tile([128, 1], fp32, tag="std")
                nc.scalar.sqrt(std[:], varr[:])
                rstd = spool.tile([128, 1], fp32, tag="rstd")
                nc.vector.reciprocal(rstd[:], std[:])
                nb = spool.tile([128, 1], fp32, tag="nb")
                nc.vector.scalar_tensor_tensor(out=nb[:], in0=mean[:], scalar=-1.0,
                                               in1=rstd[:], op0=OP.mult, op1=OP.mult)
                xh = bpool.tile([128, 256], fp32, tag="xh")
                nc.scalar.activation(xh[:], o_t[:], AF.Identity, scale=rstd[:], bias=nb[:])
                gz = bpool.tile([128, 256], fp32, tag="gz")
                nc.vector.tensor_tensor(out=gz[:], in0=xh[:], in1=gam_sb[:], op=OP.mult)
                zt = bpool.tile([128, 256], fp32, tag="zt")
                nc.vector.tensor_tensor(out=zt[:], in0=gz[:], in1=bet_sb[:], op=OP.add)
                yt = bpool.tile([128, 256], fp32, tag="yt")
                nc.scalar.activation(yt[:], zt[:], AF.Relu)
                nc.sync.dma_start(out=out[r0:r0 + 128, :], in_=yt[:])
    return nc



def _split_waits(bir_bytes):
    """Walrus on this stack only accepts one sync-wait per instruction.
    Split extra waits into standalone single-wait NoOps on the same
    engine queue (exact raw-bass semantics: in-order queue stalls)."""
    import orjson
    m = orjson.loads(bir_bytes)
    counter = [0]

    def proc(obj):
        if isinstance(obj, dict):
            for k, v in obj.items():
                if k == "instructions" and isinstance(v, list):
                    new = []
                    for ins in v:
                        si = ins.get("sync_info")
                        waits = (si or {}).get("on_wait") or []
                        lim = 0 if ins.get("opcode") == "ISA" else 1
                        if si and len(waits) > lim:
                            keep = waits[-lim:] if lim else []
                            for w in (waits[:-1] if lim else waits):
                                counter[0] += 1
                                new.append({
                                    "name": f"I-wsplit-{counter[0]}",
                                    "opcode": "EventSemaphore",
                                    "engine": ins.get("engine"),
                                    "ins": [], "outs": [],
                                    "debug": ins.get("debug"),
                                    "sync_info": {"on_update": [],
                                                  "on_wait": [w]},
                                })
                            si["on_wait"] = keep
                        new.append(ins)
                        proc(ins)
                    obj[k] = new
                else:
                    proc(v)
        elif isinstance(obj, list):
            for x in obj:
                proc(x)

    proc(m)
    return orjson.dumps(m)


def kernel(**inputs):
    global LAST_RESULT
    import os
    from concourse.bass_utils import run_bass_kernel_spmd

    feat = np.ascontiguousarray(np.asarray(inputs["feat"], dtype=np.float32))
    Wr = np.asarray(inputs["Wr"], dtype=np.float32)
    br = np.asarray(inputs["br"], dtype=np.float32)
    rl = np.asarray(inputs["rel_attn_l"], dtype=np.float32)
    rr = np.asarray(inputs["rel_attn_r"], dtype=np.float32)
    g = np.asarray(inputs["ln_gamma"], dtype=np.float32)
    b = np.asarray(inputs["ln_beta"], dtype=np.float32)

    # per-node "has incoming edge" masks (graph structure -> node sharding prep)
    mask = np.ones((N, 4), np.float32)
    for m in range(M):
        dst = np.asarray(inputs[f"dst{m}"])
        mask[:, m] = np.bincount(dst, minlength=N) > 0

    # fold rel_attn into the weight matrix:  vl = feat @ (Wr @ RLbd) + br@RLbd
    rl_bd = np.zeros((256, 4), np.float32)
    rr_bd = np.zeros((256, 4), np.float32)
    for h in range(H):
        rl_bd[h * C:(h + 1) * C, h] = rl[h]
        rr_bd[h * C:(h + 1) * C, h] = rr[h]
    A = np.concatenate([Wr @ rl_bd, Wr @ rr_bd], axis=1)          # [256, 8]
    abias = np.concatenate([br @ rl_bd, br @ rr_bd])              # [8]

    const = np.zeros((128, 1628), np.float32)
    const[:, 0:256] = Wr[0:128]
    const[:, 256:512] = Wr[128:256]
    const[:, 512:520] = A[0:128]
    const[:, 520:528] = A[128:256]
    const[:, 528:784] = g
    const[:, 784:1040] = b
    const[0, 1040:1296] = br
    const[0, 1296:1304] = abias
    const[0, 1304:1432] = 1.0

    key = "nc"
    if key not in _CACHE:
        nc0 = _build()
        _orig = nc0.to_json_bytes
        nc0.to_json_bytes = lambda: _split_waits(_orig())
        _CACHE[key] = nc0
    nc = _CACHE[key]

    in_maps = []
    for s in range(NCORES):
        fs = np.zeros((RPAD, 256), np.float32)
        fs[:RPC] = feat[s * RPC:(s + 1) * RPC]
        # featT[p, k, j] = fs[j, k*128 + p]
        ftT = np.ascontiguousarray(fs.T.reshape(2, 128, RPAD).transpose(1, 0, 2))
        mk = np.ones((RPAD, 4), np.float32)
        mk[:RPC] = mask[s * RPC:(s + 1) * RPC]
        cs = const.copy()
        cs[:, 1432:1628] = mk.reshape(NT, 128, 4).transpose(1, 0, 2).reshape(128, NT * 4)
        in_maps.append({"featT": ftT, "constd": cs})

    trace = bool(int(os.environ.get("KERNEL_TRACE", "0")))
    res = run_bass_kernel_spmd(nc, in_maps, list(range(NCORES)), trace=trace)
    LAST_RESULT = res
    outs = [res.results[s]["out"][:RPC] for s in range(NCORES)]
    return np.concatenate(outs, axis=0)



# revision 18
# speedup vs baseline: 4.7419x; 4.7419x over previous
"""LATTE GNN forward on 8 Trainium2 NeuronCores — v2.

Math (same collapse as baseline): per-edge message is v[dst], and the
segment-softmax weights over each dst's incoming edges sum to 1, so
    h_m[n] = v[n] * mask_m[n],  mask_m[n] = [n has an incoming edge in rel m]
    v      = feat @ Wr + br
    vl[n,h] = v[n,h,:].rel_attn_l[h]   (folded: feat @ (Wr @ RLbd))
    vr[n,h] = v[n,h,:].rel_attn_r[h]
    logit[n,r,h] = lrelu(vl + mask_r*vr);  beta = softmax over h (per r)
    s[n,h] = sum_r mask_r[n] * beta[n,r,h]   (mask_3 = 1)
    y      = relu(LN(v * s) * gamma + ln_beta)

v2 engine plan (vs 372us baseline):
  - bf16 matmuls (4x PE rate), whole featT resident in SBUF (4 big DMAs)
  - lv = feat@A in a tiny PE pre-pass into one persistent PSUM bank
  - softmax chain batched over 24/28-tile chunks on V (+one big exp on S)
  - LN stats via bn_stats (mean+var in one V pass per 2 tiles)
  - rstd = exp(-0.5*ln(var+eps)) so S only ever uses the
    natural_log_exp table set -> ONE ACT_TABLE_LOAD total
  - LN tail fused to one S op per tile: y = Relu(rstd*o - mu*rstd), bf16 out
Node-sharded 6250 rows/core, padded to 6656 = 52*128.
"""

import numpy as np

N, D, H, C, M = 50000, 256, 4, 64, 3
NCORES = 8
RPC = N // NCORES          # 6250 rows per core
NT = 52                    # 128-row tiles per core
RPAD = NT * 128            # 6656
NG = 13                    # groups of 4 tiles
EPS = 1e-5
PH_CHUNKS = [(0, 24), (24, 52)]                 # phase-2 tile ranges
ST_CHUNKS = [(t, min(t + 8, NT)) for t in range(0, NT, 8)]  # stats/F3 ranges

_CACHE = {}
LAST_RESULT = None


def _build(has_bias=False, has_affine=False):
    import concourse.bass as bass
    import concourse.mybir as mybir
    from concourse.tile import TileContext

    fp32 = mybir.dt.float32
    bf16 = mybir.dt.bfloat16
    AF = mybir.ActivationFunctionType
    OP = mybir.AluOpType
    AX = mybir.AxisListType

    nc = bass.Bass()
    featTd = nc.declare_dram_parameter("featT", [128, 2, RPAD], bf16, isOutput=False)
    wd = nc.declare_dram_parameter("wd", [128, 2, 264], bf16, isOutput=False)
    mkd = nc.declare_dram_parameter("mkd", [128, NT * 4], fp32, isOutput=False)
    # extras only used on the general path (biases / affine LN):
    # layout: [0:256) gamma, [256:512) beta, row0 [512:768) br,
    #         row0 [768:776) abias, row0 [776:904) ones
    exd = nc.declare_dram_parameter("exd", [128, 904], fp32, isOutput=False)
    outd = nc.declare_dram_parameter("out", [128, NT, 256], bf16, isOutput=True)

    QROWS = RPAD // 4      # featT DMA granularity (1664 rows)

    with TileContext(nc) as tc:
        with (
            tc.tile_pool(name="const", bufs=1) as cpool,
            tc.tile_pool(name="work", bufs=1) as wpool,
            tc.tile_pool(name="p2", bufs=2) as p2pool,
            tc.tile_pool(name="stat", bufs=2) as stpool,
            tc.tile_pool(name="o", bufs=5) as opool,
            tc.tile_pool(name="y", bufs=3) as ypool,
            tc.tile_pool(name="psv", bufs=3, space="PSUM") as pvpool,
            tc.tile_pool(name="pslv", bufs=1, space="PSUM") as plpool,
        ):
            ft_sb = cpool.tile([128, 2, RPAD], bf16, tag="ft")
            w_sb = cpool.tile([128, 2, 264], bf16, tag="w")
            mk_sb = cpool.tile([128, NT, 4], fp32, tag="mk")
            ex_sb = cpool.tile([128, 904], fp32, tag="exd")
            warm = cpool.tile([128, 1], fp32, tag="warm")
            epsc = cpool.tile([128, 1], fp32, tag="epsc")
            nc.gpsimd.memset(epsc[:], EPS)

            nc.gpsimd.dma_start(out=w_sb[:], in_=wd[:])
            nc.gpsimd.dma_start(
                out=mk_sb[:].rearrange("p t r -> p (t r)"), in_=mkd[:])
            if has_affine or has_bias:
                nc.gpsimd.dma_start(out=ex_sb[:], in_=exd[:])
            for q in range(4):
                nc.sync.dma_start(
                    out=ft_sb[:, :, q * QROWS:(q + 1) * QROWS],
                    in_=featTd[:, :, q * QROWS:(q + 1) * QROWS])

            # prewarm the (single) activation table set while DMAs run
            nc.scalar.activation(warm[:], mk_sb[:, 0, 0:1], AF.Exp)

            lv_ps = plpool.tile([128, NT, 8], fp32, tag="lv")
            lvl_sb = wpool.tile([128, NT, 4], fp32, tag="lvl")
            lvr_sb = wpool.tile([128, NT, 4], fp32, tag="lvr")
            s4_sb = wpool.tile([128, NT, 4], fp32, tag="s4")
            st6 = wpool.tile([128, NT, 6], fp32, tag="st6")
            rstd = wpool.tile([128, NT], fp32, tag="rstd")
            nb = wpool.tile([128, NT], fp32, tag="nb")

            def lv_prepass(t0, t1):
                for t in range(t0, t1):
                    r0 = t * 128
                    nc.tensor.matmul(lv_ps[:, t, :], ft_sb[:, 0, r0:r0 + 128],
                                     w_sb[:, 0, 256:264], start=True, stop=not has_bias)
                    nc.tensor.matmul(lv_ps[:, t, :], ft_sb[:, 1, r0:r0 + 128],
                                     w_sb[:, 1, 256:264], start=False,
                                     stop=not has_bias)
                    if has_bias:
                        # abias broadcast to all 128 rows via ones-row matmul
                        nc.tensor.matmul(lv_ps[:, t, :], ex_sb[0:1, 776:904],
                                         ex_sb[0:1, 768:776], start=False, stop=True)

            def phase2(ci):
                # layout [p, r, t, h] so every AP stays within 2 free dims
                t0, t1 = PH_CHUNKS[ci]
                tn = t1 - t0
                nc.scalar.copy(lvl_sb[:, t0:t1, :], lv_ps[:, t0:t1, 0:4])
                nc.scalar.copy(lvr_sb[:, t0:t1, :], lv_ps[:, t0:t1, 4:8])
                # vl/vr broadcast over r: [p, 1->4r, (t h)]
                vl3 = lvl_sb[:, t0:t1, :].rearrange("p t h -> p (t h)") \
                    .unsqueeze(1).broadcast_to((128, 4, tn * 4))
                vr3 = lvr_sb[:, t0:t1, :].rearrange("p t h -> p (t h)") \
                    .unsqueeze(1).broadcast_to((128, 4, tn * 4))
                # mask transposed to r-major then expanded over h
                mkT = p2pool.tile([128, 4, tn], fp32, tag="mkT", name="mkT")
                nc.vector.tensor_copy(
                    out=mkT[:], in_=mk_sb[:, t0:t1, :].rearrange("p t r -> p r t"))
                mkx = p2pool.tile([128, 4 * tn, 4], fp32, tag="mkx", name="mkx")
                nc.vector.tensor_copy(
                    out=mkx[:],
                    in_=mkT[:].rearrange("p r t -> p (r t)").unsqueeze(2)
                        .broadcast_to((128, 4 * tn, 4)))
                lg = p2pool.tile([128, 4, tn, 4], fp32, tag="lg", name="lg")
                lgs = lg[:].rearrange("p r t h -> p r (t h)")
                lgf = lg[:].rearrange("p r t h -> p (r t h)")
                nc.vector.scalar_tensor_tensor(
                    out=lgs,
                    in0=mkx[:].rearrange("p rt h -> p (rt h)")
                        .rearrange("p (r th) -> p r th", r=4),
                    scalar=1.0, in1=vr3, op0=OP.bypass, op1=OP.mult)
                nc.vector.tensor_tensor(out=lgs, in0=lgs, in1=vl3, op=OP.add)
                lr = p2pool.tile([128, 4, tn, 4], fp32, tag="lr", name="lr")
                lrf = lr[:].rearrange("p r t h -> p (r t h)")
                nc.vector.scalar_tensor_tensor(out=lrf, in0=lgf, scalar=0.2,
                                               in1=lgf, op0=OP.mult, op1=OP.max)
                ext = p2pool.tile([128, 4, tn, 4], fp32, tag="ext", name="ext")
                exf = ext[:].rearrange("p r t h -> p (r t h)")
                ex3 = ext[:].rearrange("p r t h -> p (r t) h")
                nc.scalar.activation(exf, lrf, AF.Exp)
                den = p2pool.tile([128, 4 * tn], fp32, tag="den", name="den")
                nc.vector.tensor_reduce(out=den[:], in_=ex3, axis=AX.X, op=OP.add)
                rden = p2pool.tile([128, 4 * tn], fp32, tag="rden", name="rden")
                nc.vector.reciprocal(rden[:], den[:])
                mrd = p2pool.tile([128, 4 * tn], fp32, tag="mrd", name="mrd")
                nc.vector.tensor_tensor(
                    out=mrd[:], in0=mkT[:].rearrange("p r t -> p (r t)"),
                    in1=rden[:], op=OP.mult)
                wex = p2pool.tile([128, 4, tn, 4], fp32, tag="wex", name="wex")
                wex3 = wex[:].rearrange("p r t h -> p (r t) h")
                nc.vector.scalar_tensor_tensor(
                    out=wex3, in0=ex3, scalar=1.0,
                    in1=mrd[:].unsqueeze(2).broadcast_to((128, 4 * tn, 4)),
                    op0=OP.bypass, op1=OP.mult)
                s4ab = p2pool.tile([128, 2, tn, 4], fp32, tag="s4ab", name="s4ab")
                s4abs = s4ab[:].rearrange("p r t h -> p r (t h)")
                nc.vector.tensor_tensor(
                    out=s4abs,
                    in0=wex[:, 0:2].rearrange("p r t h -> p r (t h)"),
                    in1=wex[:, 2:4].rearrange("p r t h -> p r (t h)"),
                    op=OP.add)
                nc.vector.tensor_tensor(
                    out=s4_sb[:, t0:t1, :].rearrange("p t h -> p (t h)"),
                    in0=s4ab[:, 0:1].rearrange("p r t h -> p (r t h)"),
                    in1=s4ab[:, 1:2].rearrange("p r t h -> p (r t h)"),
                    op=OP.add)

            def stats_chunk(t0, t1):
                tn = t1 - t0
                sl = st6[:, t0:t1, :]
                m2 = stpool.tile([128, 8], fp32, tag="m2", name="m2")[:, 0:tn]
                dd = stpool.tile([128, 8], fp32, tag="dd", name="dd")[:, 0:tn]
                cv = stpool.tile([128, 8], fp32, tag="cv", name="cv")[:, 0:tn]
                d2 = stpool.tile([128, 8], fp32, tag="d2", name="d2")[:, 0:tn]
                v256 = stpool.tile([128, 8], fp32, tag="v256", name="v256")[:, 0:tn]
                lnv = stpool.tile([128, 8], fp32, tag="lnv", name="lnv")[:, 0:tn]
                nc.vector.tensor_tensor(out=m2, in0=sl[:, :, 1], in1=sl[:, :, 4],
                                        op=OP.add)
                nc.vector.tensor_tensor(out=dd, in0=sl[:, :, 1], in1=sl[:, :, 4],
                                        op=OP.subtract)
                nc.vector.tensor_tensor(out=cv, in0=sl[:, :, 2], in1=sl[:, :, 5],
                                        op=OP.add)
                nc.vector.tensor_tensor(out=d2, in0=dd, in1=dd, op=OP.mult)
                nc.vector.scalar_tensor_tensor(out=v256, in0=d2, scalar=64.0,
                                               in1=cv, op0=OP.mult, op1=OP.add)
                nc.scalar.activation(lnv, v256, AF.Ln, scale=1.0 / 256.0,
                                     bias=epsc[:])
                nc.scalar.activation(rstd[:, t0:t1], lnv, AF.Exp, scale=-0.5)
                nc.vector.scalar_tensor_tensor(out=nb[:, t0:t1], in0=m2,
                                               scalar=-0.5, in1=rstd[:, t0:t1],
                                               op0=OP.mult, op1=OP.mult)

            # ---- emission ----
            lv_prepass(0, PH_CHUNKS[0][1])
            phase2(0)

            gam_bc = ex_sb[:, 0:256]
            bet_bc = ex_sb[:, 256:512]

            done_lvB = False
            for g in range(NG):
                tg = 4 * g
                if g == 2 and not done_lvB:
                    lv_prepass(PH_CHUNKS[0][1], NT)
                    phase2(1)
                    done_lvB = True
                v_ps = pvpool.tile([128, 4, 256], fp32, tag="v")
                for i in range(4):
                    r0 = (tg + i) * 128
                    nc.tensor.matmul(v_ps[:, i, :], ft_sb[:, 0, r0:r0 + 128],
                                     w_sb[:, 0, 0:256], start=True, stop=False)
                    nc.tensor.matmul(v_ps[:, i, :], ft_sb[:, 1, r0:r0 + 128],
                                     w_sb[:, 1, 0:256], start=False,
                                     stop=not has_bias)
                    if has_bias:
                        nc.tensor.matmul(v_ps[:, i, :], ex_sb[0:1, 776:904],
                                         ex_sb[0:1, 512:768], start=False, stop=True)
                o_t = opool.tile([128, 4, 256], bf16, tag="o")
                _OTILES[g] = o_t
                for hh in range(4):
                    nc.vector.scalar_tensor_tensor(
                        out=o_t[:, :, hh * 64:(hh + 1) * 64],
                        in0=v_ps[:, :, hh * 64:(hh + 1) * 64],
                        scalar=1.0,
                        in1=s4_sb[:, tg:tg + 4, hh:hh + 1]
                            .broadcast_to((128, 4, 64)),
                        op0=OP.bypass, op1=OP.mult)
                for i in range(4):
                    nc.vector.bn_stats(st6[:, tg + i, :], o_t[:, i, :])

                # stats + F3 + out-DMA for every completed 8-tile chunk
                for (c0, c1) in ST_CHUNKS:
                    if c1 == tg + 4:
                        stats_chunk(c0, c1)
                        for gg in range(c0 // 4, c1 // 4):
                            y_t = ypool.tile([128, 4, 256], bf16, tag="y")
                            og = _OTILES[gg]
                            for i in range(4):
                                t = 4 * gg + i
                                if not has_affine:
                                    nc.scalar.activation(
                                        y_t[:, i, :], og[:, i, :], AF.Relu,
                                        scale=rstd[:, t:t + 1], bias=nb[:, t:t + 1])
                                else:
                                    z_t = ypool.tile([128, 256], fp32, tag="z")
                                    nc.scalar.activation(
                                        z_t[:], og[:, i, :], AF.Identity,
                                        scale=rstd[:, t:t + 1], bias=nb[:, t:t + 1])
                                    gz = ypool.tile([128, 256], fp32, tag="gz")
                                    nc.vector.tensor_tensor(
                                        out=gz[:], in0=z_t[:], in1=gam_bc,
                                        op=OP.mult)
                                    zb = ypool.tile([128, 256], fp32, tag="zb")
                                    nc.vector.tensor_tensor(
                                        out=zb[:], in0=gz[:], in1=bet_bc, op=OP.add)
                                    nc.scalar.activation(y_t[:, i, :], zb[:],
                                                         AF.Relu)
                            nc.sync.dma_start(out=outd[:, 4 * gg:4 * gg + 4, :],
                                              in_=y_t[:])

    return nc


_OTILES = {}


def _split_waits(bir_bytes):
    """Walrus on this stack only accepts one sync-wait per instruction.
    Split extra waits into standalone single-wait NoOps on the same
    engine queue (exact raw-bass semantics: in-order queue stalls)."""
    import orjson
    m = orjson.loads(bir_bytes)
    counter = [0]

    def proc(obj):
        if isinstance(obj, dict):
            for k, v in obj.items():
                if k == "instructions" and isinstance(v, list):
                    new = []
                    for ins in v:
                        si = ins.get("sync_info")
                        waits = (si or {}).get("on_wait") or []
                        lim = 0 if ins.get("opcode") == "ISA" else 1
                        if si and len(waits) > lim:
                            keep = waits[-lim:] if lim else []
                            for w in (waits[:-1] if lim else waits):
                                counter[0] += 1
                                new.append({
                                    "name": f"I-wsplit-{counter[0]}",
                                    "opcode": "EventSemaphore",
                                    "engine": ins.get("engine"),
                                    "ins": [], "outs": [],
                                    "debug": ins.get("debug"),
                                    "sync_info": {"on_update": [],
                                                  "on_wait": [w]},
                                })
                            si["on_wait"] = keep
                        new.append(ins)
                        proc(ins)
                    obj[k] = new
                else:
                    proc(v)
        elif isinstance(obj, list):
            for x in obj:
                proc(x)

    proc(m)
    return orjson.dumps(m)


def kernel(**inputs):
    global LAST_RESULT
    import os
    import ml_dtypes
    from concourse.bass_utils import run_bass_kernel_spmd

    feat = np.asarray(inputs["feat"], dtype=np.float32)
    Wr = np.asarray(inputs["Wr"], dtype=np.float32)
    br = np.asarray(inputs["br"], dtype=np.float32)
    rl = np.asarray(inputs["rel_attn_l"], dtype=np.float32)
    rr = np.asarray(inputs["rel_attn_r"], dtype=np.float32)
    g = np.asarray(inputs["ln_gamma"], dtype=np.float32)
    b = np.asarray(inputs["ln_beta"], dtype=np.float32)

    mask = np.ones((N, 4), np.float32)
    for m in range(M):
        dst = np.asarray(inputs[f"dst{m}"])
        mask[:, m] = np.bincount(dst, minlength=N) > 0

    # fold rel_attn into the weight matrix: vl = feat @ (Wr @ RLbd) (+br terms)
    rl_bd = np.zeros((256, 4), np.float32)
    rr_bd = np.zeros((256, 4), np.float32)
    for h in range(H):
        rl_bd[h * C:(h + 1) * C, h] = rl[h]
        rr_bd[h * C:(h + 1) * C, h] = rr[h]
    A = np.concatenate([Wr @ rl_bd, Wr @ rr_bd], axis=1)          # [256, 8]
    abias = np.concatenate([br @ rl_bd, br @ rr_bd])              # [8]

    has_bias = bool(np.abs(br).max() > 0)
    has_affine = bool(np.abs(g - 1.0).max() > 0 or np.abs(b).max() > 0)

    Wfull = np.concatenate([Wr, A], axis=1)                       # [256, 264]
    # wd[p, kc, j] = Wfull[kc*128 + p, j]
    w_host = np.ascontiguousarray(
        Wfull.reshape(2, 128, 264).transpose(1, 0, 2)).astype(ml_dtypes.bfloat16)

    exd = np.zeros((128, 904), np.float32)
    exd[:, 0:256] = g
    exd[:, 256:512] = b
    exd[0, 512:768] = br
    exd[0, 768:776] = abias
    exd[0, 776:904] = 1.0

    key = (has_bias, has_affine)
    if key not in _CACHE:
        _OTILES.clear()
        nc0 = _build(has_bias=has_bias, has_affine=has_affine)
        _orig = nc0.to_json_bytes
        nc0.to_json_bytes = lambda: _split_waits(_orig())
        _CACHE[key] = nc0
    nc = _CACHE[key]

    in_maps = []
    for s in range(NCORES):
        fs = np.zeros((RPAD, 256), np.float32)
        fs[:RPC] = feat[s * RPC:(s + 1) * RPC]
        # featT[p, kc, j] = fs[j, kc*128 + p]
        ftT = np.ascontiguousarray(
            fs.T.reshape(2, 128, RPAD).transpose(1, 0, 2)).astype(ml_dtypes.bfloat16)
        mk = np.ones((RPAD, 4), np.float32)
        mk[:RPC] = mask[s * RPC:(s + 1) * RPC]
        mkh = np.ascontiguousarray(
            mk.reshape(NT, 128, 4).transpose(1, 0, 2)).reshape(128, NT * 4)
        in_maps.append({"featT": ftT, "wd": w_host, "mkd": mkh, "exd": exd})

    trace = bool(int(os.environ.get("KERNEL_TRACE", "0")))
    res = run_bass_kernel_spmd(nc, in_maps, list(range(NCORES)), trace=trace)
    LAST_RESULT = res
    outs = []
    for s in range(NCORES):
        o = np.asarray(res.results[s]["out"]).astype(np.float32)  # [128, NT, 256]
        outs.append(o.transpose(1, 0, 2).reshape(RPAD, 256)[:RPC])
    return np.concatenate(outs, axis=0)
